# revision 3
# baseline (speedup 1.0000x reference)
"""Trainium2 Bass kernel for nn_Decimate: 129-tap polyphase FIR decimation by q=4.

The reference's blocked-FFT conv is mathematically a strided valid correlation
    y[b, n] = sum_{j=0}^{128} x_ext[b, 4n + j] * k[j],   n in [0, 262144)
where x_ext = [reflect_64(x), x, zeros_64]  (length 1048704 = 128 * 8193).

Let chunk_m[p] = x_ext[128 m + p].  With n = 128 C + i,
    y[128 C + i] = sum_{s=0}^{4} sum_p chunk_{4C+s}[p] * k[128 s + p - 4 i]
(taps masked to j in [0, 128]).  This is 5 PSUM-accumulated matmuls per
512-output-chunk block:
    O[i, C] = sum_s W_s.T @ Z_s      W_s[p, i] = k[128 s + p - 4 i]  (stationary,
                                     full 128-wide; zero cols are free)
    Z_s[p, C] = chunk_{4C+s}[p]      (moving, N=512 signal columns; s=4 is
                                     Z_0 shifted one column)
Weight-stationary keeps LDWEIGHTS off the critical path (5 per 512-block vs
one per banded matmul).  bf16 in / bf16 out (gate is 2e-2; this lands ~3e-3).
Host packs chunk-deinterleaved planes, device does large plain DMAs, host
transposes the [i, C] output back to linear order.
"""

import numpy as np
import ml_dtypes

import concourse.bacc as bacc
import concourse.mybir as mybir
import concourse.tile as tile
from concourse.bass_utils import run_bass_kernel_spmd
from concourse.vector_clock import ScopedClock


class _LeanTile(tile.TileContext):
    """TileContext whose epilogue uses sem-only all-engine barriers."""

    def _drain_and_barrier(self, tick_clock, wait_clock):
        drain_inst = self.nc.sync.drain()
        wait_clock.add_sem_waits(
            drain_inst.ins, ScopedClock({None: tick_clock.global_clock}))
        self.nc.all_engine_barrier(sem_only=True)
        popped = self.nc._tile_sem_poison_stack.pop()
        assert popped is self._sem_poison
        self.nc.clear_and_free_semaphores(
            list(self.sems.allocated().values()))
        self.nc.all_engine_barrier(sem_only=True)


bf16 = ml_dtypes.bfloat16

# Problem constants (hardcoded per harness contract)
T = 1048576
NTAP = 129
Q = 4
PAD = 64
ROWS = 16
N_CORES = 8
ROWS_PER_CORE = ROWS // N_CORES          # 2
OUT = T // Q                             # 262144 outputs per row
CBLK = 128                               # elements per input chunk
NCH = 8193                               # chunks in x_ext
NCH_P = 8196                             # chunks, padded to multiple of 4
PLANE_COLS = NCH_P // 4                  # 2049 chunk-columns per plane
NCPRIME = OUT // CBLK                    # 2048 output chunks per row
SLAB_C = 512                             # output-chunk columns per slab
SLAB_W = 513                             # slab width incl. +1 halo col
N_SLABS = NCPRIME // SLAB_C              # 4 slabs per row
NPLANE = 4                               # chunk-phase planes

_PROGRAM = None


def _build_weights(k):
    """W[s, p, i] = k[128 s + p - 4 i] masked to j in [0, 128]."""
    W = np.zeros((5, 128, 128), dtype=np.float32)
    p = np.arange(128)[:, None]
    i = np.arange(128)[None, :]
    for s in range(5):
        j = 128 * s + p - 4 * i
        m = (j >= 0) & (j <= 128)
        W[s][m] = k[j[m]]
    return W


def _build_planes(x):
    """x: [B, T] fp32 -> chunk-phase planes [B, 4, 128, PLANE_COLS] fp32."""
    B = x.shape[0]
    xe = np.zeros((B, NCH_P * CBLK), dtype=np.float32)
    xe[:, PAD:PAD + T] = x
    xe[:, :PAD] = x[:, 1:PAD + 1][:, ::-1]
    ch = xe.reshape(B, PLANE_COLS, 4, CBLK)      # [B, c, g, p]
    return ch.transpose(0, 2, 3, 1)              # [B, g, p, c]


def _build_program():
    """Build the per-core Bass/Tile program (same NEFF on all 8 cores)."""
    nc = bacc.Bacc(None)
    f32 = mybir.dt.float32
    b16 = mybir.dt.bfloat16

    # xs[row, slab, p, (plane, c_local)] — per-partition contiguous 4104 B
    xs = nc.declare_dram_parameter(
        "xs", [ROWS_PER_CORE, N_SLABS, CBLK, NPLANE * SLAB_W], b16,
        isOutput=False)
    # w[p, s, i]
    w = nc.declare_dram_parameter("w", [CBLK, 5, CBLK], b16, isOutput=False)
    # y[row, i, C]  (transposed chunk layout; host untransposes)
    y = nc.declare_dram_parameter(
        "y", [ROWS_PER_CORE, CBLK, NCPRIME], b16, isOutput=True)

    with _LeanTile(nc) as tc:
        with (
            tc.tile_pool(name="wpool", bufs=1) as wpool,
            tc.tile_pool(name="xpool", bufs=4) as xpool,
            tc.tile_pool(name="opool", bufs=3) as opool,
            tc.tile_pool(name="psum", bufs=4, space="PSUM") as psum_pool,
        ):
            w_t = wpool.tile([CBLK, 5, CBLK], b16, tag="w")
            nc.scalar.dma_start(out=w_t[:], in_=w[:])

            H = NPLANE * SLAB_W // 2
            for row in range(ROWS_PER_CORE):
                for g in range(N_SLABS):
                    t = xpool.tile([CBLK, NPLANE * SLAB_W], b16, tag="xs")
                    # split the slab load in half for a faster pipeline ramp
                    nc.sync.dma_start(out=t[:, :H], in_=xs[row, g, :, :H])
                    nc.sync.dma_start(out=t[:, H:], in_=xs[row, g, :, H:])
                    O = psum_pool.tile([CBLK, SLAB_C], f32, tag="O")
                    for s in range(5):
                        pl, off = s % 4, s // 4
                        nc.tensor.matmul(
                            O[:],
                            w_t[:, s, :],
                            t[:, pl * SLAB_W + off:pl * SLAB_W + off + SLAB_C],
                            start=(s == 0), stop=(s == 4))
                    stage = opool.tile([CBLK, SLAB_C], b16, tag="stage")
                    nc.vector.tensor_copy(stage[:], O[:])
                    nc.scalar.dma_start(
                        out=y[row, :, SLAB_C * g:SLAB_C * (g + 1)],
                        in_=stage[:])
    nc.finalize()
    return nc


def _get_program():
    global _PROGRAM
    if _PROGRAM is None:
        _PROGRAM = _build_program()
    return _PROGRAM


def _prepare_in_maps(x, k):
    planes = _build_planes(np.ascontiguousarray(x, dtype=np.float32))
    ph = planes.astype(bf16)                     # [B, g, p, c]

    # pack [B, slab, p, (plane, c_local)] with per-partition contiguity
    B = x.shape[0]
    xsv = np.zeros((B, N_SLABS, CBLK, NPLANE, SLAB_W), dtype=bf16)
    for blk in range(N_SLABS):
        sl = slice(SLAB_C * blk, SLAB_C * blk + SLAB_W)
        for g in range(4):
            xsv[:, blk, :, g, :] = ph[:, g, :, sl]
    xsv = xsv.reshape(B, N_SLABS, CBLK, NPLANE * SLAB_W)

    W = _build_weights(np.asarray(k, dtype=np.float32))
    w_t = np.ascontiguousarray(np.transpose(W.astype(bf16), (1, 0, 2)))

    in_maps = []
    for c in range(N_CORES):
        sl = slice(c * ROWS_PER_CORE, (c + 1) * ROWS_PER_CORE)
        in_maps.append({
            "xs": np.ascontiguousarray(xsv[sl]),
            "w": w_t,
        })
    return in_maps


def _run(x, k, trace=False):
    nc = _get_program()
    in_maps = _prepare_in_maps(x, k)
    res = run_bass_kernel_spmd(nc, in_maps, list(range(N_CORES)), trace=trace)
    # y[row, i, C] -> y_lin[row, 128 C + i]
    outs = [np.asarray(r["y"]).astype(np.float32).transpose(0, 2, 1)
            .reshape(ROWS_PER_CORE, OUT) for r in res.results]
    out = np.concatenate(outs, axis=0).reshape(ROWS, OUT)
    return out, res


def kernel(x, kernel, q):
    assert int(q) == Q and x.shape == (ROWS, T) and kernel.shape == (NTAP,)
    out, _ = _run(np.asarray(x), np.asarray(kernel), trace=False)
    return out


def kernel_traced(x, kernel, q):
    """Like kernel() but returns (out, BassKernelResults) with HW profile."""
    out, res = _run(np.asarray(x), np.asarray(kernel), trace=True)
    return out, res


# revision 6
# speedup vs baseline: 1.0268x; 1.0268x over previous
"""Trainium2 Bass kernel for nn_Decimate: 129-tap polyphase FIR decimation by q=4.

The reference's blocked-FFT conv is mathematically a strided valid correlation
    y[b, n] = sum_{j=0}^{128} x_ext[b, 4n + j] * k[j],   n in [0, 262144)
where x_ext = [reflect_64(x), x, zeros_64]  (length 1048704 = 128 * 8193).

Let chunk_m[p] = x_ext[128 m + p].  With n = 128 C + i,
    y[128 C + i] = sum_{s=0}^{4} sum_p chunk_{4C+s}[p] * k[128 s + p - 4 i]
(taps masked to j in [0, 128]).  This is 5 PSUM-accumulated matmuls per
512-output-chunk block:
    O[i, C] = sum_s W_s.T @ Z_s      W_s[p, i] = k[128 s + p - 4 i]  (stationary,
                                     full 128-wide; zero cols are free)
    Z_s[p, C] = chunk_{4C+s}[p]      (moving, N=512 signal columns; s=4 is
                                     Z_0 shifted one column)
Weight-stationary keeps LDWEIGHTS off the critical path (5 per 512-block vs
one per banded matmul).  bf16 in / bf16 out (gate is 2e-2; this lands ~3e-3).
Host packs chunk-deinterleaved planes, device does large plain DMAs, host
transposes the [i, C] output back to linear order.
"""

import numpy as np
import ml_dtypes

import concourse.bacc as bacc
import concourse.mybir as mybir
import concourse.tile as tile
from concourse.bass_utils import run_bass_kernel_spmd
from concourse.vector_clock import ScopedClock


class _LeanTile(tile.TileContext):
    """TileContext whose epilogue uses sem-only all-engine barriers."""

    def _drain_and_barrier(self, tick_clock, wait_clock):
        drain_inst = self.nc.sync.drain()
        wait_clock.add_sem_waits(
            drain_inst.ins, ScopedClock({None: tick_clock.global_clock}))
        self.nc.all_engine_barrier(sem_only=True)
        popped = self.nc._tile_sem_poison_stack.pop()
        assert popped is self._sem_poison
        self.nc.clear_and_free_semaphores(
            list(self.sems.allocated().values()))
        self.nc.all_engine_barrier(sem_only=True)


bf16 = ml_dtypes.bfloat16

# Problem constants (hardcoded per harness contract)
T = 1048576
NTAP = 129
Q = 4
PAD = 64
ROWS = 16
N_CORES = 8
ROWS_PER_CORE = ROWS // N_CORES          # 2
OUT = T // Q                             # 262144 outputs per row
CBLK = 128                               # elements per input chunk
NCH = 8193                               # chunks in x_ext
NCH_P = 8196                             # chunks, padded to multiple of 4
PLANE_COLS = NCH_P // 4                  # 2049 chunk-columns per plane
NCPRIME = OUT // CBLK                    # 2048 output chunks per row
SLAB_C = 512                             # output-chunk columns per slab
SLAB_W = 513                             # slab width incl. +1 halo col
N_SLABS = NCPRIME // SLAB_C              # 4 slabs per row
NPLANE = 4                               # chunk-phase planes

_PROGRAM = None


def _build_weights(k):
    """W[s, p, i] = k[128 s + p - 4 i] masked to j in [0, 128]."""
    W = np.zeros((5, 128, 128), dtype=np.float32)
    p = np.arange(128)[:, None]
    i = np.arange(128)[None, :]
    for s in range(5):
        j = 128 * s + p - 4 * i
        m = (j >= 0) & (j <= 128)
        W[s][m] = k[j[m]]
    return W


def _build_planes(x):
    """x: [B, T] fp32 -> chunk-phase planes [B, 4, 128, PLANE_COLS] fp32."""
    B = x.shape[0]
    xe = np.zeros((B, NCH_P * CBLK), dtype=np.float32)
    xe[:, PAD:PAD + T] = x
    xe[:, :PAD] = x[:, 1:PAD + 1][:, ::-1]
    ch = xe.reshape(B, PLANE_COLS, 4, CBLK)      # [B, c, g, p]
    return ch.transpose(0, 2, 3, 1)              # [B, g, p, c]


def _build_program():
    """Build the per-core Bass/Tile program (same NEFF on all 8 cores)."""
    nc = bacc.Bacc(None)
    f32 = mybir.dt.float32
    b16 = mybir.dt.bfloat16

    # xs[row, slab, p, (plane, c_local)] — per-partition contiguous 4104 B
    xs = nc.declare_dram_parameter(
        "xs", [ROWS_PER_CORE, N_SLABS, CBLK, NPLANE * SLAB_W], b16,
        isOutput=False)
    # w[p, s, i]
    w = nc.declare_dram_parameter("w", [CBLK, 5, CBLK], b16, isOutput=False)
    # y[row, i, C]  (transposed chunk layout; host untransposes)
    y = nc.declare_dram_parameter(
        "y", [ROWS_PER_CORE, CBLK, NCPRIME], b16, isOutput=True)

    with _LeanTile(nc) as tc:
        with (
            tc.tile_pool(name="wpool", bufs=1) as wpool,
            tc.tile_pool(name="xpool", bufs=5) as xpool,
            tc.tile_pool(name="opool", bufs=3) as opool,
            tc.tile_pool(name="psum", bufs=4, space="PSUM") as psum_pool,
        ):
            w_t = wpool.tile([CBLK, 5, CBLK], b16, tag="w")
            nc.scalar.dma_start(out=w_t[:], in_=w[:])

            H = NPLANE * SLAB_W // 2
            for row in range(ROWS_PER_CORE):
                for g in range(N_SLABS):
                    t = xpool.tile([CBLK, NPLANE * SLAB_W], b16, tag="xs")
                    # split the slab load in half for a faster pipeline ramp
                    nc.sync.dma_start(out=t[:, :H], in_=xs[row, g, :, :H])
                    nc.sync.dma_start(out=t[:, H:], in_=xs[row, g, :, H:])
                    O = psum_pool.tile([CBLK, SLAB_C], f32, tag="O")
                    # Col-tiled banded waves: W_s is nonzero only on i-band
                    # s=0:[0,32) 1:[0,64) 2:[32,96) 3:[64,128) 4:[96,128).
                    # Wave A {0, 2a, 2b, 4} covers every partition exactly
                    # once (start=True each; 64-wide tiles must be
                    # 64-aligned, so s=2 splits); wave B {1,3} accumulates.
                    # Disjoint col-groups run concurrently in the PE array.
                    for s, lo, m, st, sp in ((0, 0, 32, True, False),
                                             (2, 32, 32, True, False),
                                             (2, 64, 32, True, False),
                                             (4, 96, 32, True, False),
                                             (1, 0, 64, False, False),
                                             (3, 64, 64, False, True)):
                        pl, off = s % 4, s // 4
                        nc.tensor.matmul(
                            O[lo:lo + m, :],
                            w_t[:, s, lo:lo + m],
                            t[:, pl * SLAB_W + off:pl * SLAB_W + off + SLAB_C],
                            start=st, stop=sp,
                            tile_position=(0, lo))
                    stage = opool.tile([CBLK, SLAB_C], b16, tag="stage")
                    nc.vector.tensor_copy(stage[:], O[:])
                    nc.scalar.dma_start(
                        out=y[row, :, SLAB_C * g:SLAB_C * (g + 1)],
                        in_=stage[:])
    nc.finalize()
    return nc


def _get_program():
    global _PROGRAM
    if _PROGRAM is None:
        _PROGRAM = _build_program()
    return _PROGRAM


def _prepare_in_maps(x, k):
    planes = _build_planes(np.ascontiguousarray(x, dtype=np.float32))
    ph = planes.astype(bf16)                     # [B, g, p, c]

    # pack [B, slab, p, (plane, c_local)] with per-partition contiguity
    B = x.shape[0]
    xsv = np.zeros((B, N_SLABS, CBLK, NPLANE, SLAB_W), dtype=bf16)
    for blk in range(N_SLABS):
        sl = slice(SLAB_C * blk, SLAB_C * blk + SLAB_W)
        for g in range(4):
            xsv[:, blk, :, g, :] = ph[:, g, :, sl]
    xsv = xsv.reshape(B, N_SLABS, CBLK, NPLANE * SLAB_W)

    W = _build_weights(np.asarray(k, dtype=np.float32))
    w_t = np.ascontiguousarray(np.transpose(W.astype(bf16), (1, 0, 2)))

    in_maps = []
    for c in range(N_CORES):
        sl = slice(c * ROWS_PER_CORE, (c + 1) * ROWS_PER_CORE)
        in_maps.append({
            "xs": np.ascontiguousarray(xsv[sl]),
            "w": w_t,
        })
    return in_maps


def _run(x, k, trace=False):
    nc = _get_program()
    in_maps = _prepare_in_maps(x, k)
    res = run_bass_kernel_spmd(nc, in_maps, list(range(N_CORES)), trace=trace)
    # y[row, i, C] -> y_lin[row, 128 C + i]
    outs = [np.asarray(r["y"]).astype(np.float32).transpose(0, 2, 1)
            .reshape(ROWS_PER_CORE, OUT) for r in res.results]
    out = np.concatenate(outs, axis=0).reshape(ROWS, OUT)
    return out, res


def kernel(x, kernel, q):
    assert int(q) == Q and x.shape == (ROWS, T) and kernel.shape == (NTAP,)
    out, _ = _run(np.asarray(x), np.asarray(kernel), trace=False)
    return out


def kernel_traced(x, kernel, q):
    """Like kernel() but returns (out, BassKernelResults) with HW profile."""
    out, res = _run(np.asarray(x), np.asarray(kernel), trace=True)
    return out, res
